# revision 1
# baseline (speedup 1.0000x reference)
"""MoE (noisy top-2 routing, dense expert stack) on 8 Trainium2 NeuronCores.

Strategy: expert-parallel with host-side routing as the sharding step. The
host computes the noisy gating in fp64 (bit-robust reproduction of the
reference's fp32 top-2 selection), ships each core exactly the tokens routed
to its expert (padded to a uniform tile count so all 8 cores run the same
SPMD program), plus the per-token top-2 softmax combine weight — the device
runs nothing but the expert FFN, in fp16 (fp16 inputs, fp32 PSUM
accumulation: ~4e-4 end-to-end error against the fp32 reference).

Both weight matrices live in SBUF for the whole kernel (fp16 halves their
footprint), so the only per-tile DMA traffic is the x tile in and the y tile
out. Layer 1 emits h transposed (h-major) straight into SBUF as fp16, so it
chains into layer 2 as the stationary operand with no transpose.

DMA queue discipline (a consumer waits for every DMA issued earlier on the
same engine queue): the SP queue carries only x tiles, issued in consumption
order — the next tile's prefetch goes out mid-layer-2, after the current
tile's first output group. The Activation HWDGE queue carries the persistent
tensors in first-use order, then alternates y stores with SP.

The host scatter-adds the (at most 2) pre-weighted output rows per token —
the "all-reduce of the weighted combine" of the expert-parallel sharding,
done as part of unsharding. Per-core compute is the routed ~2/8 of the dense
reference instead of all 8 experts on all tokens.
"""

import sys

sys.path.insert(0, "/opt/trn_rl_repo")

import numpy as np

import concourse.bass as bass
import concourse.mybir as mybir
import concourse.tile as tile
from concourse import bacc
from concourse.bass_utils import run_bass_kernel_spmd

N_CORES = 8
N, D, H, E = 8192, 1024, 2048, 8
P = 128
KD = D // P                 # 8  k-chunks over D
MH = H // P                 # 16 h-chunks

F32 = mybir.dt.float32
F16 = mybir.dt.float16
ALU = mybir.AluOpType
ACT_F = mybir.ActivationFunctionType


def _build(slots, repeat=1):
    """SPMD program for one core = one expert over `slots` routed tokens."""
    assert slots % P == 0 and slots % 512 in (0, 256, 384)
    widths = [512] * (slots // 512)
    if slots % 512:
        widths.append(slots % 512)

    nc = bacc.Bacc(None, target_bir_lowering=False, debug=False)

    xT = nc.dram_tensor("xT", [D, slots], F16, kind="ExternalInput")
    w1c = nc.dram_tensor("w1c", [D, H], F16, kind="ExternalInput")
    w2c = nc.dram_tensor("w2c", [H, D], F16, kind="ExternalInput")
    b1c = nc.dram_tensor("b1c", [H], F32, kind="ExternalInput")
    wvd = nc.dram_tensor("wvd", [slots], F32, kind="ExternalInput")
    yc = nc.dram_tensor("yc", [slots, D], F32, kind="ExternalOutput")

    with tile.TileContext(nc) as tc:
        with (
            tc.tile_pool(name="persist", bufs=1) as persist,
            tc.tile_pool(name="xs", bufs=2) as xs,
            tc.tile_pool(name="hs", bufs=2) as hs,
            tc.tile_pool(name="yws", bufs=3) as yws,
            tc.tile_pool(name="ph", bufs=4, space="PSUM") as ph,
            tc.tile_pool(name="py", bufs=2, space="PSUM") as py,
        ):
            def x_tile():
                return xs.tile([P, KD, 512], F16, tag="xg", name="xg")

            def load_x(xtile, ss, TW):
                # two kd-half DMAs: the first half-contraction of layer 1
                # only waits on the first half of the tile
                src = xT[:, ss].rearrange("(kd p) t -> p kd t", p=P)
                nc.sync.dma_start(xtile[:, : KD // 2, :TW], src[:, : KD // 2, :])
                nc.sync.dma_start(xtile[:, KD // 2 :, :TW], src[:, KD // 2 :, :])

            cur = x_tile()
            load_x(cur, slice(0, widths[0]), widths[0])
            # W1 as eight column-eighth tiles so the first layer-1 matmuls
            # depend only on the first eighth's DMA (~1.5us less startup).
            W1_sb = [
                persist.tile([P, KD, H // 8], F16, name=f"W1e{q}") for q in range(8)
            ]
            b1_sb = persist.tile([P, MH], F32)
            for q in range(8):
                qs = slice(q * (H // 8), (q + 1) * (H // 8))
                nc.scalar.dma_start(
                    W1_sb[q][:], w1c[:, qs].rearrange("(kd p) h -> p kd h", p=P)
                )
                if q == 1:
                    nc.scalar.dma_start(
                        b1_sb[:], b1c.rearrange("(m p) -> p m", p=P)
                    )
            wcol = persist.tile([P, slots // P], F32)
            nc.scalar.dma_start(wcol[:], wvd.rearrange("(c p) -> p c", p=P))
            # W2 as two nh-half tiles, in layer-2 consumption order.
            W2_sb = [
                persist.tile([P, MH, D // 2], F16, name=f"W2h{i}") for i in range(2)
            ]
            for i in range(2):
                ns = slice(i * (D // 2), (i + 1) * (D // 2))
                nc.scalar.dma_start(
                    W2_sb[i][:], w2c[:, ns].rearrange("(kh p) d -> p kh d", p=P)
                )

            nt = len(widths)
            for _rep in range(repeat):
                for ti, TW in enumerate(widths):
                    base = sum(widths[:ti])
                    nch = TW // P
                    xg = cur
                    nti = (ti + 1) % nt
                    nxt = None
                    if _rep < repeat - 1 or ti < nt - 1:
                        nxt = x_tile()

                    # layer 1: hT = relu(W1^T @ x + b1), h on partitions,
                    # cast to fp16 by the activation itself. hT is two
                    # half-tensors (m 0-7 / 8-15) so layer 2's first k-chunks
                    # only depend on the first half.
                    hth = [
                        hs.tile([P, MH // 2, 512], F16, tag=f"hth{i}", name=f"hth{i}")
                        for i in range(2)
                    ]
                    for m in range(MH):
                        h_ps = ph.tile([P, 512], F32, tag="hps")
                        ms = slice((m % 2) * P, (m % 2 + 1) * P)
                        for kd in range(KD):
                            nc.tensor.matmul(
                                h_ps[:, :TW],
                                W1_sb[m // 2][:, kd, ms],
                                xg[:, kd, :TW],
                                start=(kd == 0),
                                stop=(kd == KD - 1),
                            )
                        nc.vector.tensor_scalar(
                            hth[m // 8][:, m % 8, :TW],
                            h_ps[:, :TW],
                            b1_sb[:, m : m + 1],
                            0.0,
                            ALU.add,
                            ALU.max,
                        )

                    # layer 2: y = hT^T @ W2 + b2, then scale rows by the
                    # host-computed top-2 softmax weight and store. The two
                    # nh output halves run interleaved per kh so consecutive
                    # matmuls share the stationary hT chunk (one weight load
                    # feeds 2x512 moving rows).
                    for c4 in range(nch):
                        cs = slice(c4 * P, (c4 + 1) * P)
                        y_ps = [
                            py.tile([P, 512], F32, tag=f"yps{i}", name=f"yps{i}")
                            for i in range(2)
                        ]
                        for kh in range(MH):
                            hsl = hth[kh // 8][:, kh % 8, cs]
                            for nh in range(2):
                                nc.tensor.matmul(
                                    y_ps[nh][:],
                                    hsl,
                                    W2_sb[nh][:, kh, :],
                                    start=(kh == 0),
                                    stop=(kh == MH - 1),
                                )
                        if c4 == 0 and nxt is not None:
                            nbase = sum(widths[:nti])
                            load_x(nxt, slice(nbase, nbase + widths[nti]), widths[nti])
                        ch = base // P + c4
                        for nh in range(2):
                            ns = slice(nh * 512, (nh + 1) * 512)
                            yw = yws.tile([P, 512], F32, tag="yw")
                            nc.vector.tensor_scalar(
                                yw[:], y_ps[nh][:], wcol[:, ch : ch + 1],
                                None, ALU.mult,
                            )
                            st_eng = nc.sync if nh else nc.scalar
                            st_eng.dma_start(
                                yc[base + c4 * P : base + (c4 + 1) * P, ns],
                                yw[:],
                            )
                    if nxt is not None:
                        cur = nxt

    nc.compile()
    return nc


_NC_CACHE = {}


def _get_nc(slots, repeat=1):
    key = (slots, repeat)
    if key not in _NC_CACHE:
        _NC_CACHE[key] = _build(slots, repeat)
    return _NC_CACHE[key]


def prepare(x, W1, b1, W2, b2, Wg, bg, noise):
    """Host-side routing/sharding: fp64 noisy top-2, per-expert token lists,
    fp16 casts, per-core input maps, and the scatter-add spec."""
    x = np.ascontiguousarray(np.asarray(x, dtype=np.float32))
    noise = np.asarray(noise, dtype=np.float32)
    W1 = np.asarray(W1, dtype=np.float32)
    b1 = np.asarray(b1, dtype=np.float32)
    W2 = np.asarray(W2, dtype=np.float32)
    b2 = np.asarray(b2, dtype=np.float32)
    Wg = np.asarray(Wg, dtype=np.float32)
    bg = np.asarray(bg, dtype=np.float32)

    noisy = (
        x.astype(np.float64) @ Wg.astype(np.float64)
        + bg.astype(np.float64)
        + 0.1 * noise.astype(np.float64)
    )
    top2 = np.argsort(-noisy, axis=1)[:, :2]

    tok_lists = [np.nonzero((top2 == e).any(axis=1))[0] for e in range(E)]
    # Cap device slots at 2048 (perfect 512-wide tiles): overloaded experts'
    # excess tokens (a few hundred at most, expert counts concentrate around
    # N*K/E = 2048) are computed exactly on the host during unshard instead
    # of padding every core up to the worst expert.
    cap = 1792
    max_count = min(cap, max(len(t) for t in tok_lists))
    slots = max(512, ((max_count + P - 1) // P) * P)
    if slots % 512 == P:
        slots += P

    x16 = x.astype(np.float16)

    def tok_weights(toks, e):
        other = np.where(top2[toks, 0] == e, top2[toks, 1], top2[toks, 0])
        diff = noisy[toks, e] - noisy[toks, other]
        return (1.0 / (1.0 + np.exp(-diff))).astype(np.float32)

    in_maps = []
    gathers = []
    host_work = []
    b2w = []
    for e in range(E):
        toks = tok_lists[e][:slots]
        extra = tok_lists[e][slots:]
        if len(extra):
            host_work.append((e, extra, tok_weights(extra, e)))
        cnt = len(toks)
        padded = np.zeros(slots, dtype=np.int64)
        padded[:cnt] = toks
        wv = np.zeros(slots, dtype=np.float32)
        if cnt:
            wv[:cnt] = tok_weights(toks, e)
        in_maps.append(
            {
                "xT": np.ascontiguousarray(x16[padded].T),
                "w1c": W1[e].astype(np.float16),
                "w2c": W2[e].astype(np.float16),
                "b1c": np.ascontiguousarray(b1[e]),
                "wvd": wv,
            }
        )
        gathers.append(toks)
        b2w.append((toks, wv[: len(toks)], b2[e]))

    host_rows = []
    for e, extra, w in host_work:
        h = np.maximum(x[extra] @ W1[e] + b1[e], 0.0)
        host_rows.append((extra, w[:, None] * (h @ W2[e] + b2[e])))
    return in_maps, (gathers, host_rows, b2w), slots


def combine(results, gathers):
    """Unshard: scatter-add each core's pre-weighted rows into the output,
    plus the host-computed rows of over-capacity experts."""
    gathers, host_rows, b2w = gathers
    out = np.zeros((N, D), dtype=np.float32)
    for e in range(E):
        toks = gathers[e]
        out[toks] += results[e]["yc"][: len(toks)]
    for toks, rows in host_rows:
        out[toks] += rows
    for toks, wv, b2row in b2w:
        if len(toks) and np.any(b2row):
            out[toks] += wv[:, None] * b2row[None, :]
    return out


def kernel(x, W1, b1, W2, b2, Wg, bg, noise, **_ignored):
    in_maps, gathers, slots = prepare(x, W1, b1, W2, b2, Wg, bg, noise)
    nc = _get_nc(slots)
    res = run_bass_kernel_spmd(nc, in_maps, core_ids=list(range(N_CORES)))
    return combine(res.results, gathers)



# revision 2
# speedup vs baseline: 1.0165x; 1.0165x over previous
"""MoE (noisy top-2 routing, dense expert stack) on 8 Trainium2 NeuronCores.

Strategy: expert-parallel with host-side routing as the sharding step. The
host computes the noisy gating in fp64 (bit-robust reproduction of the
reference's fp32 top-2 selection), ships each core exactly the tokens routed
to its expert (padded to a uniform tile count so all 8 cores run the same
SPMD program), plus the per-token top-2 softmax combine weight — the device
runs nothing but the expert FFN, in fp16 (fp16 inputs, fp32 PSUM
accumulation: ~4e-4 end-to-end error against the fp32 reference).

Both weight matrices live in SBUF for the whole kernel (fp16 halves their
footprint), so the only per-tile DMA traffic is the x tile in and the y tile
out. Layer 1 emits h transposed (h-major) straight into SBUF as fp16, so it
chains into layer 2 as the stationary operand with no transpose.

DMA queue discipline (a consumer waits for every DMA issued earlier on the
same engine queue): the SP queue carries only x tiles, issued in consumption
order — the next tile's prefetch goes out mid-layer-2, after the current
tile's first output group. The Activation HWDGE queue carries the persistent
tensors in first-use order, then alternates y stores with SP.

The host scatter-adds the (at most 2) pre-weighted output rows per token —
the "all-reduce of the weighted combine" of the expert-parallel sharding,
done as part of unsharding. Per-core compute is the routed ~2/8 of the dense
reference instead of all 8 experts on all tokens.
"""

import sys

sys.path.insert(0, "/opt/trn_rl_repo")

import numpy as np

import concourse.bass as bass
import concourse.mybir as mybir
import concourse.tile as tile
from concourse import bacc
from concourse.bass_utils import run_bass_kernel_spmd

N_CORES = 8
N, D, H, E = 8192, 1024, 2048, 8
P = 128
KD = D // P                 # 8  k-chunks over D
MH = H // P                 # 16 h-chunks

F32 = mybir.dt.float32
F16 = mybir.dt.float16
ALU = mybir.AluOpType
ACT_F = mybir.ActivationFunctionType


def _build(slots, repeat=1):
    """SPMD program for one core = one expert over `slots` routed tokens."""
    assert slots % P == 0 and slots % 512 in (0, 256, 384)
    widths = [512] * (slots // 512)
    if slots % 512:
        widths.append(slots % 512)

    nc = bacc.Bacc(None, target_bir_lowering=False, debug=False)

    xT = nc.dram_tensor("xT", [D, slots], F16, kind="ExternalInput")
    w1c = nc.dram_tensor("w1c", [D, H], F16, kind="ExternalInput")
    w2c = nc.dram_tensor("w2c", [H, D], F16, kind="ExternalInput")
    b1c = nc.dram_tensor("b1c", [H], F32, kind="ExternalInput")
    wvd = nc.dram_tensor("wvd", [slots], F32, kind="ExternalInput")
    yc = nc.dram_tensor("yc", [slots, D], F32, kind="ExternalOutput")

    with tile.TileContext(nc) as tc:
        with (
            tc.tile_pool(name="persist", bufs=1) as persist,
            tc.tile_pool(name="xs", bufs=2) as xs,
            tc.tile_pool(name="hs", bufs=2) as hs,
            tc.tile_pool(name="yws", bufs=3) as yws,
            tc.tile_pool(name="ph", bufs=4, space="PSUM") as ph,
            tc.tile_pool(name="py", bufs=2, space="PSUM") as py,
        ):
            def x_tile():
                return xs.tile([P, KD, 512], F16, tag="xg", name="xg")

            def load_x(xtile, ss, TW):
                # two kd-half DMAs: the first half-contraction of layer 1
                # only waits on the first half of the tile
                src = xT[:, ss].rearrange("(kd p) t -> p kd t", p=P)
                nc.sync.dma_start(xtile[:, : KD // 2, :TW], src[:, : KD // 2, :])
                nc.sync.dma_start(xtile[:, KD // 2 :, :TW], src[:, KD // 2 :, :])

            cur = x_tile()
            # First tile in four kd-pair pieces: the first matmul gates on
            # one 128KB piece instead of half the tile.
            src0 = xT[:, 0 : widths[0]].rearrange("(kd p) t -> p kd t", p=P)
            for piece in range(4):
                ks = slice(2 * piece, 2 * piece + 2)
                nc.sync.dma_start(cur[:, ks, : widths[0]], src0[:, ks, :])
            # W1 as eight column-eighth tiles so the first layer-1 matmuls
            # depend only on the first eighth's DMA (~1.5us less startup);
            # the first eighth itself lands in two kd-half pieces.
            W1_sb = [
                persist.tile([P, KD, H // 8], F16, name=f"W1e{q}") for q in range(8)
            ]
            b1_sb = persist.tile([P, MH], F32)
            for q in range(8):
                qs = slice(q * (H // 8), (q + 1) * (H // 8))
                srcq = w1c[:, qs].rearrange("(kd p) h -> p kd h", p=P)
                if q == 0:
                    nc.scalar.dma_start(W1_sb[0][:, : KD // 2, :], srcq[:, : KD // 2, :])
                    nc.scalar.dma_start(W1_sb[0][:, KD // 2 :, :], srcq[:, KD // 2 :, :])
                else:
                    nc.scalar.dma_start(W1_sb[q][:], srcq)
                if q == 1:
                    nc.scalar.dma_start(
                        b1_sb[:], b1c.rearrange("(m p) -> p m", p=P)
                    )
            wcol = persist.tile([P, slots // P], F32)
            nc.scalar.dma_start(wcol[:], wvd.rearrange("(c p) -> p c", p=P))
            # W2 as two nh-half tiles, in layer-2 consumption order.
            W2_sb = [
                persist.tile([P, MH, D // 2], F16, name=f"W2h{i}") for i in range(2)
            ]
            for i in range(2):
                ns = slice(i * (D // 2), (i + 1) * (D // 2))
                nc.scalar.dma_start(
                    W2_sb[i][:], w2c[:, ns].rearrange("(kh p) d -> p kh d", p=P)
                )

            nt = len(widths)
            for _rep in range(repeat):
                for ti, TW in enumerate(widths):
                    base = sum(widths[:ti])
                    nch = TW // P
                    xg = cur
                    nti = (ti + 1) % nt
                    nxt = None
                    if _rep < repeat - 1 or ti < nt - 1:
                        nxt = x_tile()

                    # layer 1: hT = relu(W1^T @ x + b1), h on partitions,
                    # cast to fp16 by the activation itself. hT is two
                    # half-tensors (m 0-7 / 8-15) so layer 2's first k-chunks
                    # only depend on the first half.
                    hth = [
                        hs.tile([P, MH // 2, 512], F16, tag=f"hth{i}", name=f"hth{i}")
                        for i in range(2)
                    ]
                    for m in range(MH):
                        h_ps = ph.tile([P, 512], F32, tag="hps")
                        ms = slice((m % 2) * P, (m % 2 + 1) * P)
                        for kd in range(KD):
                            nc.tensor.matmul(
                                h_ps[:, :TW],
                                W1_sb[m // 2][:, kd, ms],
                                xg[:, kd, :TW],
                                start=(kd == 0),
                                stop=(kd == KD - 1),
                            )
                        nc.vector.tensor_scalar(
                            hth[m // 8][:, m % 8, :TW],
                            h_ps[:, :TW],
                            b1_sb[:, m : m + 1],
                            0.0,
                            ALU.add,
                            ALU.max,
                        )

                    # layer 2: y = hT^T @ W2 + b2, then scale rows by the
                    # host-computed top-2 softmax weight and store. The two
                    # nh output halves run interleaved per kh so consecutive
                    # matmuls share the stationary hT chunk (one weight load
                    # feeds 2x512 moving rows).
                    for c4 in range(nch):
                        cs = slice(c4 * P, (c4 + 1) * P)
                        y_ps = [
                            py.tile([P, 512], F32, tag=f"yps{i}", name=f"yps{i}")
                            for i in range(2)
                        ]
                        for kh in range(MH):
                            hsl = hth[kh // 8][:, kh % 8, cs]
                            for nh in range(2):
                                nc.tensor.matmul(
                                    y_ps[nh][:],
                                    hsl,
                                    W2_sb[nh][:, kh, :],
                                    start=(kh == 0),
                                    stop=(kh == MH - 1),
                                )
                        if c4 == 0 and nxt is not None:
                            nbase = sum(widths[:nti])
                            load_x(nxt, slice(nbase, nbase + widths[nti]), widths[nti])
                        ch = base // P + c4
                        for nh in range(2):
                            ns = slice(nh * 512, (nh + 1) * 512)
                            yw = yws.tile([P, 512], F32, tag="yw")
                            nc.vector.tensor_scalar(
                                yw[:], y_ps[nh][:], wcol[:, ch : ch + 1],
                                None, ALU.mult,
                            )
                            st_eng = nc.sync if nh else nc.scalar
                            st_eng.dma_start(
                                yc[base + c4 * P : base + (c4 + 1) * P, ns],
                                yw[:],
                            )
                    if nxt is not None:
                        cur = nxt

    nc.compile()
    return nc


_NC_CACHE = {}


def _get_nc(slots, repeat=1):
    key = (slots, repeat)
    if key not in _NC_CACHE:
        _NC_CACHE[key] = _build(slots, repeat)
    return _NC_CACHE[key]


def prepare(x, W1, b1, W2, b2, Wg, bg, noise):
    """Host-side routing/sharding: fp64 noisy top-2, per-expert token lists,
    fp16 casts, per-core input maps, and the scatter-add spec."""
    x = np.ascontiguousarray(np.asarray(x, dtype=np.float32))
    noise = np.asarray(noise, dtype=np.float32)
    W1 = np.asarray(W1, dtype=np.float32)
    b1 = np.asarray(b1, dtype=np.float32)
    W2 = np.asarray(W2, dtype=np.float32)
    b2 = np.asarray(b2, dtype=np.float32)
    Wg = np.asarray(Wg, dtype=np.float32)
    bg = np.asarray(bg, dtype=np.float32)

    noisy = (
        x.astype(np.float64) @ Wg.astype(np.float64)
        + bg.astype(np.float64)
        + 0.1 * noise.astype(np.float64)
    )
    top2 = np.argsort(-noisy, axis=1)[:, :2]

    tok_lists = [np.nonzero((top2 == e).any(axis=1))[0] for e in range(E)]
    # Cap device slots at 2048 (perfect 512-wide tiles): overloaded experts'
    # excess tokens (a few hundred at most, expert counts concentrate around
    # N*K/E = 2048) are computed exactly on the host during unshard instead
    # of padding every core up to the worst expert.
    cap = 1792
    max_count = min(cap, max(len(t) for t in tok_lists))
    slots = max(512, ((max_count + P - 1) // P) * P)
    if slots % 512 == P:
        slots += P

    x16 = x.astype(np.float16)

    def tok_weights(toks, e):
        other = np.where(top2[toks, 0] == e, top2[toks, 1], top2[toks, 0])
        diff = noisy[toks, e] - noisy[toks, other]
        return (1.0 / (1.0 + np.exp(-diff))).astype(np.float32)

    in_maps = []
    gathers = []
    host_work = []
    b2w = []
    for e in range(E):
        toks = tok_lists[e][:slots]
        extra = tok_lists[e][slots:]
        if len(extra):
            host_work.append((e, extra, tok_weights(extra, e)))
        cnt = len(toks)
        padded = np.zeros(slots, dtype=np.int64)
        padded[:cnt] = toks
        wv = np.zeros(slots, dtype=np.float32)
        if cnt:
            wv[:cnt] = tok_weights(toks, e)
        in_maps.append(
            {
                "xT": np.ascontiguousarray(x16[padded].T),
                "w1c": W1[e].astype(np.float16),
                "w2c": W2[e].astype(np.float16),
                "b1c": np.ascontiguousarray(b1[e]),
                "wvd": wv,
            }
        )
        gathers.append(toks)
        b2w.append((toks, wv[: len(toks)], b2[e]))

    host_rows = []
    for e, extra, w in host_work:
        h = np.maximum(x[extra] @ W1[e] + b1[e], 0.0)
        host_rows.append((extra, w[:, None] * (h @ W2[e] + b2[e])))
    return in_maps, (gathers, host_rows, b2w), slots


def combine(results, gathers):
    """Unshard: scatter-add each core's pre-weighted rows into the output,
    plus the host-computed rows of over-capacity experts."""
    gathers, host_rows, b2w = gathers
    out = np.zeros((N, D), dtype=np.float32)
    for e in range(E):
        toks = gathers[e]
        out[toks] += results[e]["yc"][: len(toks)]
    for toks, rows in host_rows:
        out[toks] += rows
    for toks, wv, b2row in b2w:
        if len(toks) and np.any(b2row):
            out[toks] += wv[:, None] * b2row[None, :]
    return out


def kernel(x, W1, b1, W2, b2, Wg, bg, noise, **_ignored):
    in_maps, gathers, slots = prepare(x, W1, b1, W2, b2, Wg, bg, noise)
    nc = _get_nc(slots)
    res = run_bass_kernel_spmd(nc, in_maps, core_ids=list(range(N_CORES)))
    return combine(res.results, gathers)



# revision 4
# speedup vs baseline: 1.0187x; 1.0022x over previous
"""MoE (noisy top-2 routing, dense expert stack) on 8 Trainium2 NeuronCores.

Strategy: expert-parallel with host-side routing as the sharding step. The
host computes the noisy gating in fp64 (bit-robust reproduction of the
reference's fp32 top-2 selection), ships each core exactly the tokens routed
to its expert (padded to a uniform tile count so all 8 cores run the same
SPMD program), plus the per-token top-2 softmax combine weight — the device
runs nothing but the expert FFN, in fp16 (fp16 inputs, fp32 PSUM
accumulation: ~4e-4 end-to-end error against the fp32 reference).

Both weight matrices live in SBUF for the whole kernel (fp16 halves their
footprint), so the only per-tile DMA traffic is the x tile in and the y tile
out. Layer 1 emits h transposed (h-major) straight into SBUF as fp16, so it
chains into layer 2 as the stationary operand with no transpose.

DMA queue discipline (a consumer waits for every DMA issued earlier on the
same engine queue): the SP queue carries only x tiles, issued in consumption
order — the next tile's prefetch goes out mid-layer-2, after the current
tile's first output group. The Activation HWDGE queue carries the persistent
tensors in first-use order, then alternates y stores with SP.

The host scatter-adds the (at most 2) pre-weighted output rows per token —
the "all-reduce of the weighted combine" of the expert-parallel sharding,
done as part of unsharding. Per-core compute is the routed ~2/8 of the dense
reference instead of all 8 experts on all tokens.
"""

import sys

sys.path.insert(0, "/opt/trn_rl_repo")

import numpy as np

import concourse.bass as bass
import concourse.mybir as mybir
import concourse.tile as tile
from concourse import bacc
from concourse.bass_utils import run_bass_kernel_spmd

N_CORES = 8
N, D, H, E = 8192, 1024, 2048, 8
P = 128
KD = D // P                 # 8  k-chunks over D
MH = H // P                 # 16 h-chunks

F32 = mybir.dt.float32
F16 = mybir.dt.float16
ALU = mybir.AluOpType
ACT_F = mybir.ActivationFunctionType


def _build(slots, repeat=1):
    """SPMD program for one core = one expert over `slots` routed tokens."""
    assert slots % P == 0 and slots % 512 in (0, 256, 384)
    widths = [512] * (slots // 512)
    if slots % 512:
        widths.append(slots % 512)

    nc = bacc.Bacc(None, target_bir_lowering=False, debug=False)

    xT = nc.dram_tensor("xT", [D, slots], F16, kind="ExternalInput")
    w1c = nc.dram_tensor("w1c", [D, H], F16, kind="ExternalInput")
    w2c = nc.dram_tensor("w2c", [H, D], F16, kind="ExternalInput")
    b1c = nc.dram_tensor("b1c", [H], F32, kind="ExternalInput")
    wvd = nc.dram_tensor("wvd", [slots], F32, kind="ExternalInput")
    yc = nc.dram_tensor("yc", [slots, D], F32, kind="ExternalOutput")

    with tile.TileContext(nc) as tc:
        with (
            tc.tile_pool(name="persist", bufs=1) as persist,
            tc.tile_pool(name="xs", bufs=2) as xs,
            tc.tile_pool(name="hs", bufs=2) as hs,
            tc.tile_pool(name="yws", bufs=3) as yws,
            tc.tile_pool(name="ph", bufs=4, space="PSUM") as ph,
            tc.tile_pool(name="py", bufs=2, space="PSUM") as py,
        ):
            def x_tile():
                return xs.tile([P, KD, 512], F16, tag="xg", name="xg")

            def load_x(xtile, ss, TW):
                # two kd-half DMAs: the first half-contraction of layer 1
                # only waits on the first half of the tile. x rides the Pool
                # HWDGE queue so the sync queue carries only y stores.
                src = xT[:, ss].rearrange("(kd p) t -> p kd t", p=P)
                nc.gpsimd.dma_start(xtile[:, : KD // 2, :TW], src[:, : KD // 2, :])
                nc.gpsimd.dma_start(xtile[:, KD // 2 :, :TW], src[:, KD // 2 :, :])

            cur = x_tile()
            # First tile in four kd-pair pieces: the first matmul gates on
            # one 128KB piece instead of half the tile.
            src0 = xT[:, 0 : widths[0]].rearrange("(kd p) t -> p kd t", p=P)
            for piece in range(4):
                ks = slice(2 * piece, 2 * piece + 2)
                nc.gpsimd.dma_start(cur[:, ks, : widths[0]], src0[:, ks, :])
            # W1 as eight column-eighth tiles so the first layer-1 matmuls
            # depend only on the first eighth's DMA (~1.5us less startup);
            # the first eighth itself lands in two kd-half pieces.
            W1_sb = [
                persist.tile([P, KD, H // 8], F16, name=f"W1e{q}") for q in range(8)
            ]
            b1_sb = persist.tile([P, MH], F32)
            for q in range(8):
                qs = slice(q * (H // 8), (q + 1) * (H // 8))
                srcq = w1c[:, qs].rearrange("(kd p) h -> p kd h", p=P)
                if q == 0:
                    nc.scalar.dma_start(W1_sb[0][:, : KD // 2, :], srcq[:, : KD // 2, :])
                    nc.scalar.dma_start(W1_sb[0][:, KD // 2 :, :], srcq[:, KD // 2 :, :])
                else:
                    nc.scalar.dma_start(W1_sb[q][:], srcq)
                if q == 1:
                    nc.scalar.dma_start(
                        b1_sb[:], b1c.rearrange("(m p) -> p m", p=P)
                    )
            wcol = persist.tile([P, slots // P], F32)
            nc.scalar.dma_start(wcol[:], wvd.rearrange("(c p) -> p c", p=P))
            # W2 as two nh-half tiles, in layer-2 consumption order.
            W2_sb = [
                persist.tile([P, MH, D // 2], F16, name=f"W2h{i}") for i in range(2)
            ]
            for i in range(2):
                ns = slice(i * (D // 2), (i + 1) * (D // 2))
                nc.scalar.dma_start(
                    W2_sb[i][:], w2c[:, ns].rearrange("(kh p) d -> p kh d", p=P)
                )

            nt = len(widths)
            for _rep in range(repeat):
                for ti, TW in enumerate(widths):
                    base = sum(widths[:ti])
                    nch = TW // P
                    xg = cur
                    nti = (ti + 1) % nt
                    nxt = None
                    if _rep < repeat - 1 or ti < nt - 1:
                        nxt = x_tile()

                    # layer 1: hT = relu(W1^T @ x + b1), h on partitions,
                    # cast to fp16 by the activation itself. hT is two
                    # half-tensors (m 0-7 / 8-15) so layer 2's first k-chunks
                    # only depend on the first half.
                    hth = [
                        hs.tile([P, MH // 2, 512], F16, tag=f"hth{i}", name=f"hth{i}")
                        for i in range(2)
                    ]
                    for m in range(MH):
                        h_ps = ph.tile([P, 512], F32, tag="hps")
                        ms = slice((m % 2) * P, (m % 2 + 1) * P)
                        for kd in range(KD):
                            nc.tensor.matmul(
                                h_ps[:, :TW],
                                W1_sb[m // 2][:, kd, ms],
                                xg[:, kd, :TW],
                                start=(kd == 0),
                                stop=(kd == KD - 1),
                            )
                        # bias+relu on the ACT engine: keeps the DVE free for
                        # the y scales and halves traffic on each engine's
                        # in-order queue (measured ~6us/rep faster than DVE).
                        nc.scalar.activation(
                            hth[m // 8][:, m % 8, :TW],
                            h_ps[:, :TW],
                            ACT_F.Relu,
                            b1_sb[:, m : m + 1],
                        )

                    # layer 2: y = hT^T @ W2 + b2, then scale rows by the
                    # host-computed top-2 softmax weight and store. The two
                    # nh output halves run interleaved per kh so consecutive
                    # matmuls share the stationary hT chunk (one weight load
                    # feeds 2x512 moving rows).
                    for c4 in range(nch):
                        cs = slice(c4 * P, (c4 + 1) * P)
                        y_ps = [
                            py.tile([P, 512], F32, tag=f"yps{i}", name=f"yps{i}")
                            for i in range(2)
                        ]
                        for kh in range(MH):
                            hsl = hth[kh // 8][:, kh % 8, cs]
                            for nh in range(2):
                                nc.tensor.matmul(
                                    y_ps[nh][:],
                                    hsl,
                                    W2_sb[nh][:, kh, :],
                                    start=(kh == 0),
                                    stop=(kh == MH - 1),
                                )
                        if c4 == 0 and nxt is not None:
                            nbase = sum(widths[:nti])
                            load_x(nxt, slice(nbase, nbase + widths[nti]), widths[nti])
                        ch = base // P + c4
                        for nh in range(2):
                            ns = slice(nh * 512, (nh + 1) * 512)
                            yw = yws.tile([P, 512], F32, tag="yw")
                            nc.vector.tensor_scalar(
                                yw[:], y_ps[nh][:], wcol[:, ch : ch + 1],
                                None, ALU.mult,
                            )
                            st_eng = nc.sync if nh else nc.scalar
                            st_eng.dma_start(
                                yc[base + c4 * P : base + (c4 + 1) * P, ns],
                                yw[:],
                            )
                    if nxt is not None:
                        cur = nxt

    nc.compile()
    return nc


_NC_CACHE = {}


def _get_nc(slots, repeat=1):
    key = (slots, repeat)
    if key not in _NC_CACHE:
        _NC_CACHE[key] = _build(slots, repeat)
    return _NC_CACHE[key]


def prepare(x, W1, b1, W2, b2, Wg, bg, noise):
    """Host-side routing/sharding: fp64 noisy top-2, per-expert token lists,
    fp16 casts, per-core input maps, and the scatter-add spec."""
    x = np.ascontiguousarray(np.asarray(x, dtype=np.float32))
    noise = np.asarray(noise, dtype=np.float32)
    W1 = np.asarray(W1, dtype=np.float32)
    b1 = np.asarray(b1, dtype=np.float32)
    W2 = np.asarray(W2, dtype=np.float32)
    b2 = np.asarray(b2, dtype=np.float32)
    Wg = np.asarray(Wg, dtype=np.float32)
    bg = np.asarray(bg, dtype=np.float32)

    noisy = (
        x.astype(np.float64) @ Wg.astype(np.float64)
        + bg.astype(np.float64)
        + 0.1 * noise.astype(np.float64)
    )
    top2 = np.argsort(-noisy, axis=1)[:, :2]

    tok_lists = [np.nonzero((top2 == e).any(axis=1))[0] for e in range(E)]
    # Cap device slots at 2048 (perfect 512-wide tiles): overloaded experts'
    # excess tokens (a few hundred at most, expert counts concentrate around
    # N*K/E = 2048) are computed exactly on the host during unshard instead
    # of padding every core up to the worst expert.
    cap = 1792
    max_count = min(cap, max(len(t) for t in tok_lists))
    slots = max(512, ((max_count + P - 1) // P) * P)
    if slots % 512 == P:
        slots += P

    x16 = x.astype(np.float16)

    def tok_weights(toks, e):
        other = np.where(top2[toks, 0] == e, top2[toks, 1], top2[toks, 0])
        diff = noisy[toks, e] - noisy[toks, other]
        return (1.0 / (1.0 + np.exp(-diff))).astype(np.float32)

    in_maps = []
    gathers = []
    host_work = []
    b2w = []
    for e in range(E):
        toks = tok_lists[e][:slots]
        extra = tok_lists[e][slots:]
        if len(extra):
            host_work.append((e, extra, tok_weights(extra, e)))
        cnt = len(toks)
        padded = np.zeros(slots, dtype=np.int64)
        padded[:cnt] = toks
        wv = np.zeros(slots, dtype=np.float32)
        if cnt:
            wv[:cnt] = tok_weights(toks, e)
        in_maps.append(
            {
                "xT": np.ascontiguousarray(x16[padded].T),
                "w1c": W1[e].astype(np.float16),
                "w2c": W2[e].astype(np.float16),
                "b1c": np.ascontiguousarray(b1[e]),
                "wvd": wv,
            }
        )
        gathers.append(toks)
        b2w.append((toks, wv[: len(toks)], b2[e]))

    host_rows = []
    for e, extra, w in host_work:
        h = np.maximum(x[extra] @ W1[e] + b1[e], 0.0)
        host_rows.append((extra, w[:, None] * (h @ W2[e] + b2[e])))
    return in_maps, (gathers, host_rows, b2w), slots


def combine(results, gathers):
    """Unshard: scatter-add each core's pre-weighted rows into the output,
    plus the host-computed rows of over-capacity experts."""
    gathers, host_rows, b2w = gathers
    out = np.zeros((N, D), dtype=np.float32)
    for e in range(E):
        toks = gathers[e]
        out[toks] += results[e]["yc"][: len(toks)]
    for toks, rows in host_rows:
        out[toks] += rows
    for toks, wv, b2row in b2w:
        if len(toks) and np.any(b2row):
            out[toks] += wv[:, None] * b2row[None, :]
    return out


def kernel(x, W1, b1, W2, b2, Wg, bg, noise, **_ignored):
    in_maps, gathers, slots = prepare(x, W1, b1, W2, b2, Wg, bg, noise)
    nc = _get_nc(slots)
    res = run_bass_kernel_spmd(nc, in_maps, core_ids=list(range(N_CORES)))
    return combine(res.results, gathers)



# revision 5
# speedup vs baseline: 1.0509x; 1.0316x over previous
"""MoE (noisy top-2 routing, dense expert stack) on 8 Trainium2 NeuronCores.

Strategy: expert-parallel with host-side routing as the sharding step. The
host computes the noisy gating in fp64 (bit-robust reproduction of the
reference's fp32 top-2 selection), ships each core exactly the tokens routed
to its expert (padded to a uniform tile count so all 8 cores run the same
SPMD program), plus the per-token top-2 softmax combine weight — the device
runs nothing but the expert FFN, in fp16 (fp16 inputs, fp32 PSUM
accumulation: ~4e-4 end-to-end error against the fp32 reference).

Both weight matrices live in SBUF for the whole kernel (fp16 halves their
footprint), so the only per-tile DMA traffic is the x tile in and the y tile
out. Layer 1 emits h transposed (h-major) straight into SBUF as fp16, so it
chains into layer 2 as the stationary operand with no transpose.

DMA queue discipline (a consumer waits for every DMA issued earlier on the
same engine queue): the SP queue carries only x tiles, issued in consumption
order — the next tile's prefetch goes out mid-layer-2, after the current
tile's first output group. The Activation HWDGE queue carries the persistent
tensors in first-use order, then alternates y stores with SP.

The host scatter-adds the (at most 2) pre-weighted output rows per token —
the "all-reduce of the weighted combine" of the expert-parallel sharding,
done as part of unsharding. Per-core compute is the routed ~2/8 of the dense
reference instead of all 8 experts on all tokens.
"""

import sys

sys.path.insert(0, "/opt/trn_rl_repo")

import numpy as np

import concourse.bass as bass
import concourse.mybir as mybir
import concourse.tile as tile
from concourse import bacc
from concourse.bass_utils import run_bass_kernel_spmd

N_CORES = 8
N, D, H, E = 8192, 1024, 2048, 8
P = 128
KD = D // P                 # 8  k-chunks over D
MH = H // P                 # 16 h-chunks

F32 = mybir.dt.float32
F16 = mybir.dt.float16
ALU = mybir.AluOpType
ACT_F = mybir.ActivationFunctionType


def _build(slots, repeat=1):
    """SPMD program for one core = one expert over `slots` routed tokens."""
    assert slots % P == 0 and slots % 512 in (0, 256, 384)
    widths = [512] * (slots // 512)
    if slots % 512:
        widths.append(slots % 512)

    nc = bacc.Bacc(None, target_bir_lowering=False, debug=False)

    xT = nc.dram_tensor("xT", [D, slots], F16, kind="ExternalInput")
    w1c = nc.dram_tensor("w1c", [D, H], F16, kind="ExternalInput")
    w2c = nc.dram_tensor("w2c", [H, D], F16, kind="ExternalInput")
    b1c = nc.dram_tensor("b1c", [H], F32, kind="ExternalInput")
    wvd = nc.dram_tensor("wvd", [slots], F32, kind="ExternalInput")
    yc = nc.dram_tensor("yc", [slots, D], F32, kind="ExternalOutput")

    with tile.TileContext(nc) as tc:
        with (
            tc.tile_pool(name="persist", bufs=1) as persist,
            tc.tile_pool(name="xs", bufs=2) as xs,
            tc.tile_pool(name="hs", bufs=2) as hs,
            tc.tile_pool(name="yws", bufs=3) as yws,
            tc.tile_pool(name="ph", bufs=4, space="PSUM") as ph,
            tc.tile_pool(name="py", bufs=2, space="PSUM") as py,
        ):
            def x_tile():
                return xs.tile([P, KD, 512], F16, tag="xg", name="xg")

            def load_x(xtile, ss, TW):
                # two kd-half DMAs: the first half-contraction of layer 1
                # only waits on the first half of the tile. x rides the Pool
                # HWDGE queue so the sync queue carries only y stores.
                src = xT[:, ss].rearrange("(kd p) t -> p kd t", p=P)
                nc.gpsimd.dma_start(xtile[:, : KD // 2, :TW], src[:, : KD // 2, :])
                nc.gpsimd.dma_start(xtile[:, KD // 2 :, :TW], src[:, KD // 2 :, :])

            cur = x_tile()
            # First tile in four kd-pair pieces: the first matmul gates on
            # one 128KB piece instead of half the tile.
            src0 = xT[:, 0 : widths[0]].rearrange("(kd p) t -> p kd t", p=P)
            for piece in range(4):
                ks = slice(2 * piece, 2 * piece + 2)
                nc.gpsimd.dma_start(cur[:, ks, : widths[0]], src0[:, ks, :])
            # W1 as eight column-eighth tiles so the first layer-1 matmuls
            # depend only on the first eighth's DMA (~1.5us less startup);
            # the first eighth itself lands in two kd-half pieces.
            W1_sb = [
                persist.tile([P, KD, H // 8], F16, name=f"W1e{q}") for q in range(8)
            ]
            b1_sb = persist.tile([P, MH], F32)
            for q in range(8):
                qs = slice(q * (H // 8), (q + 1) * (H // 8))
                srcq = w1c[:, qs].rearrange("(kd p) h -> p kd h", p=P)
                if q == 0:
                    nc.scalar.dma_start(W1_sb[0][:, : KD // 2, :], srcq[:, : KD // 2, :])
                    nc.scalar.dma_start(W1_sb[0][:, KD // 2 :, :], srcq[:, KD // 2 :, :])
                else:
                    nc.scalar.dma_start(W1_sb[q][:], srcq)
                if q == 1:
                    nc.scalar.dma_start(
                        b1_sb[:], b1c.rearrange("(m p) -> p m", p=P)
                    )
            wcol = persist.tile([P, slots // P], F32)
            nc.scalar.dma_start(wcol[:], wvd.rearrange("(c p) -> p c", p=P))
            # W2 as two nh-half tiles, in layer-2 consumption order.
            W2_sb = [
                persist.tile([P, MH, D // 2], F16, name=f"W2h{i}") for i in range(2)
            ]
            for i in range(2):
                ns = slice(i * (D // 2), (i + 1) * (D // 2))
                nc.scalar.dma_start(
                    W2_sb[i][:], w2c[:, ns].rearrange("(kh p) d -> p kh d", p=P)
                )

            nt = len(widths)
            for _rep in range(repeat):
                for ti, TW in enumerate(widths):
                    base = sum(widths[:ti])
                    nch = TW // P
                    xg = cur
                    nti = (ti + 1) % nt
                    nxt = None
                    if _rep < repeat - 1 or ti < nt - 1:
                        nxt = x_tile()

                    # layer 1: hT = relu(W1^T @ x + b1), h on partitions,
                    # cast to fp16 by the activation itself. hT is two
                    # half-tensors (m 0-7 / 8-15) so layer 2's first k-chunks
                    # only depend on the first half.
                    hth = [
                        hs.tile([P, MH // 2, 512], F16, tag=f"hth{i}", name=f"hth{i}")
                        for i in range(2)
                    ]
                    for m in range(MH):
                        h_ps = ph.tile([P, 512], F32, tag="hps")
                        ms = slice((m % 2) * P, (m % 2 + 1) * P)
                        for kd in range(KD):
                            nc.tensor.matmul(
                                h_ps[:, :TW],
                                W1_sb[m // 2][:, kd, ms],
                                xg[:, kd, :TW],
                                start=(kd == 0),
                                stop=(kd == KD - 1),
                            )
                        # bias+relu on the ACT engine: keeps the DVE free for
                        # the y scales and halves traffic on each engine's
                        # in-order queue (measured ~6us/rep faster than DVE).
                        nc.scalar.activation(
                            hth[m // 8][:, m % 8, :TW],
                            h_ps[:, :TW],
                            ACT_F.Relu,
                            b1_sb[:, m : m + 1],
                        )

                    # layer 2: y = hT^T @ W2 + b2, then scale rows by the
                    # host-computed top-2 softmax weight and store. The two
                    # nh output halves run interleaved per kh so consecutive
                    # matmuls share the stationary hT chunk (one weight load
                    # feeds 2x512 moving rows).
                    for c4 in range(nch):
                        cs = slice(c4 * P, (c4 + 1) * P)
                        y_ps = [
                            py.tile([P, 512], F32, tag=f"yps{i}", name=f"yps{i}")
                            for i in range(2)
                        ]
                        for kh in range(MH):
                            hsl = hth[kh // 8][:, kh % 8, cs]
                            for nh in range(2):
                                nc.tensor.matmul(
                                    y_ps[nh][:],
                                    hsl,
                                    W2_sb[nh][:, kh, :],
                                    start=(kh == 0),
                                    stop=(kh == MH - 1),
                                )
                        if c4 == 0 and nxt is not None:
                            nbase = sum(widths[:nti])
                            load_x(nxt, slice(nbase, nbase + widths[nti]), widths[nti])
                        ch = base // P + c4
                        # Both output halves scale into one [P, 1024] tile and
                        # leave in a single store: full 4KB-contiguous DRAM
                        # rows and half the descriptor count (measured
                        # ~14us/rep faster than two 512-wide stores).
                        yw = yws.tile([P, 1024], F32, tag="yw")
                        for nh in range(2):
                            nc.vector.tensor_scalar(
                                yw[:, nh * 512 : (nh + 1) * 512],
                                y_ps[nh][:], wcol[:, ch : ch + 1],
                                None, ALU.mult,
                            )
                        st_eng = nc.sync if c4 % 2 else nc.scalar
                        st_eng.dma_start(
                            yc[base + c4 * P : base + (c4 + 1) * P, :],
                            yw[:],
                        )
                    if nxt is not None:
                        cur = nxt

    nc.compile()
    return nc


_NC_CACHE = {}


def _get_nc(slots, repeat=1):
    key = (slots, repeat)
    if key not in _NC_CACHE:
        _NC_CACHE[key] = _build(slots, repeat)
    return _NC_CACHE[key]


def prepare(x, W1, b1, W2, b2, Wg, bg, noise):
    """Host-side routing/sharding: fp64 noisy top-2, per-expert token lists,
    fp16 casts, per-core input maps, and the scatter-add spec."""
    x = np.ascontiguousarray(np.asarray(x, dtype=np.float32))
    noise = np.asarray(noise, dtype=np.float32)
    W1 = np.asarray(W1, dtype=np.float32)
    b1 = np.asarray(b1, dtype=np.float32)
    W2 = np.asarray(W2, dtype=np.float32)
    b2 = np.asarray(b2, dtype=np.float32)
    Wg = np.asarray(Wg, dtype=np.float32)
    bg = np.asarray(bg, dtype=np.float32)

    noisy = (
        x.astype(np.float64) @ Wg.astype(np.float64)
        + bg.astype(np.float64)
        + 0.1 * noise.astype(np.float64)
    )
    top2 = np.argsort(-noisy, axis=1)[:, :2]

    tok_lists = [np.nonzero((top2 == e).any(axis=1))[0] for e in range(E)]
    # Cap device slots at 2048 (perfect 512-wide tiles): overloaded experts'
    # excess tokens (a few hundred at most, expert counts concentrate around
    # N*K/E = 2048) are computed exactly on the host during unshard instead
    # of padding every core up to the worst expert.
    cap = 1792
    max_count = min(cap, max(len(t) for t in tok_lists))
    slots = max(512, ((max_count + P - 1) // P) * P)
    if slots % 512 == P:
        slots += P

    x16 = x.astype(np.float16)

    def tok_weights(toks, e):
        other = np.where(top2[toks, 0] == e, top2[toks, 1], top2[toks, 0])
        diff = noisy[toks, e] - noisy[toks, other]
        return (1.0 / (1.0 + np.exp(-diff))).astype(np.float32)

    in_maps = []
    gathers = []
    host_work = []
    b2w = []
    for e in range(E):
        toks = tok_lists[e][:slots]
        extra = tok_lists[e][slots:]
        if len(extra):
            host_work.append((e, extra, tok_weights(extra, e)))
        cnt = len(toks)
        padded = np.zeros(slots, dtype=np.int64)
        padded[:cnt] = toks
        wv = np.zeros(slots, dtype=np.float32)
        if cnt:
            wv[:cnt] = tok_weights(toks, e)
        in_maps.append(
            {
                "xT": np.ascontiguousarray(x16[padded].T),
                "w1c": W1[e].astype(np.float16),
                "w2c": W2[e].astype(np.float16),
                "b1c": np.ascontiguousarray(b1[e]),
                "wvd": wv,
            }
        )
        gathers.append(toks)
        b2w.append((toks, wv[: len(toks)], b2[e]))

    host_rows = []
    for e, extra, w in host_work:
        h = np.maximum(x[extra] @ W1[e] + b1[e], 0.0)
        host_rows.append((extra, w[:, None] * (h @ W2[e] + b2[e])))
    return in_maps, (gathers, host_rows, b2w), slots


def combine(results, gathers):
    """Unshard: scatter-add each core's pre-weighted rows into the output,
    plus the host-computed rows of over-capacity experts."""
    gathers, host_rows, b2w = gathers
    out = np.zeros((N, D), dtype=np.float32)
    for e in range(E):
        toks = gathers[e]
        out[toks] += results[e]["yc"][: len(toks)]
    for toks, rows in host_rows:
        out[toks] += rows
    for toks, wv, b2row in b2w:
        if len(toks) and np.any(b2row):
            out[toks] += wv[:, None] * b2row[None, :]
    return out


def kernel(x, W1, b1, W2, b2, Wg, bg, noise, **_ignored):
    in_maps, gathers, slots = prepare(x, W1, b1, W2, b2, Wg, bg, noise)
    nc = _get_nc(slots)
    res = run_bass_kernel_spmd(nc, in_maps, core_ids=list(range(N_CORES)))
    return combine(res.results, gathers)

